# revision 7
# baseline (speedup 1.0000x reference)
"""Mixtral MoE layer (T=1024, H=1024, I=2048, E=8, top-2) on 8 Trainium2 cores.

Strategy: token-sparse expert-parallel. The router (softmax + top-2 +
renormalize -> combine[T, E]) runs on host. Core c owns expert c's FFN and
processes only the tokens routed to expert c (on average T*K/E = 256,
padded to a fixed bucket NCAP=384; zero-padded columns contribute nothing).
Host gathers each expert's token columns of x^T (the "token all-to-all"
shard step), the device computes

    outT_c = (w2_c @ (silu(w1_c @ xg) * (w3_c @ xg))) * combine[toks_c, c]

and host scatter-adds the per-expert [H, n_c] panels back into the full
[T, H] output (the unshard step). If any expert overflows the bucket
(never for 8 experts at these sizes unless routing is degenerate), we fall
back to a dense variant: every core processes all T tokens with its
combine column, same scatter-add (toks = arange(T)).

Matmuls run as float32r (TF32-like precision, ~2.5e-4 rel err end to end,
full PE rate for moving dims >= 256). Weights are repacked on host so each
i-tile's w1/w3 lhsT blocks and w2 rows form one contiguous [128, 12KB]
DMA (~1.5 MiB per dma_start, descriptor-efficient).
"""

import os
import sys

sys.path.insert(0, "/opt/trn_rl_repo")

import numpy as np

import concourse.bacc as bacc
import concourse.tile as tile
from concourse import mybir
from concourse.bass_utils import run_bass_kernel_spmd

F32 = mybir.dt.float32
F32R = mybir.dt.float32r

T = 1024   # tokens
H = 1024   # hidden
I = 2048   # intermediate
E = 8      # experts
TOPK = 2
P = 128
NKH = H // P     # 8  h-tiles (up-proj contraction)
NI = I // P      # 16 i-tiles
NH = H // P      # 8  h-tiles (down-proj output)
N_CORES = 8
NCAP = 384       # token bucket per expert (seed-robust: mean 256, std ~14)
WCOLS = 3 * NKH * P  # packed weight row: w1 blocks | w3 blocks | w2 rows

_NC_CACHE = {}


def build_nc(ncap: int):
    if ncap in _NC_CACHE:
        return _NC_CACHE[ncap]

    # moving-operand blocks of <=512 (>=256 keeps float32r at full rate)
    nblk = (ncap + 511) // 512
    blks = []
    for b in range(nblk):
        lo = b * 512
        blks.append(slice(lo, min(lo + 512, ncap)))

    nc = bacc.Bacc(None, target_bir_lowering=False, num_devices=N_CORES)

    xg_in = nc.declare_dram_parameter("xg", [H, ncap], F32, isOutput=False)
    comb_in = nc.declare_dram_parameter("comb", [P, ncap], F32, isOutput=False)
    # per i-tile packed weights: [w1 lhsT (NKH*P) | w3 lhsT (NKH*P) | w2 rows (H)]
    wq_in = nc.declare_dram_parameter("wq", [NI, P, WCOLS], F32, isOutput=False)
    outp = nc.declare_dram_parameter("outp", [H, ncap], F32, isOutput=True)

    with tile.TileContext(nc) as tc:
        with (
            tc.tile_pool(name="persist", bufs=1) as persist,
            tc.tile_pool(name="stream", bufs=2) as stream,
            tc.tile_pool(name="psum", bufs=1, space="PSUM") as psum,
        ):
            # first i-tile's weights ahead of the xg block so PE starts early;
            # split so the w1 blocks (first consumers) land first
            WB = NKH * P
            wq0 = stream.tile([P, WCOLS], F32R, name="wq_0", tag="wq", bufs=4)
            nc.sync.dma_start(out=wq0[:, 0:WB], in_=wq_in[0][:, 0:WB].bitcast(F32R))

            xg_sb = []

            def load_xg(kh):
                t_ = persist.tile([P, ncap], F32R, name=f"xg_{kh}", tag=f"xg_{kh}")
                nc.sync.dma_start(
                    out=t_[:], in_=xg_in[kh * P : (kh + 1) * P, :].bitcast(F32R)
                )
                xg_sb.append(t_)

            for kh in range(3):
                load_xg(kh)
            nc.sync.dma_start(
                out=wq0[:, WB : 2 * WB], in_=wq_in[0][:, WB : 2 * WB].bitcast(F32R)
            )
            for kh in range(3, NKH):
                load_xg(kh)
            nc.sync.dma_start(
                out=wq0[:, 2 * WB :], in_=wq_in[0][:, 2 * WB :].bitcast(F32R)
            )
            comb_sb = persist.tile([P, ncap], F32, name="comb_sb", tag="comb_sb")
            nc.sync.dma_start(out=comb_sb[:], in_=comb_in[:, :])

            w2_sb = []
            act_sb = []
            poa_sb = [None] * NH
            HALF = NI // 2

            def down_mms(po, ht, ii_range, start_ii, stop_ii):
                for ii in ii_range:
                    lhsT = w2_sb[ii][:, ht * P : (ht + 1) * P]
                    for ts in blks:
                        nc.tensor.matmul(
                            po[:, ts], lhsT, act_sb[ii][:, ts],
                            start=(ii == start_ii), stop=(ii == stop_ii),
                        )

            # ---- stage 1: up-projections + SwiGLU, per i-tile; the first
            # half of the down-projection rides along with i-tiles 8..15,
            # soaking up PE slack while the weight stream is the bottleneck
            for it in range(NI):
                if it == 0:
                    wq = wq0
                else:
                    wq = stream.tile([P, WCOLS], F32R, name=f"wq_{it}", tag="wq", bufs=4)
                    nc.sync.dma_start(out=wq[:], in_=wq_in[it].bitcast(F32R))

                # stash w2 rows for stage 2 (the streaming tile gets recycled)
                w2sb = persist.tile([P, H], F32R, name=f"w2sb_{it}", tag=f"w2sb_{it}")
                nc.vector.tensor_copy(w2sb[:], wq[:, 2 * NKH * P :].bitcast(F32))
                w2_sb.append(w2sb)

                ph1 = psum.tile([P, ncap], F32, name=f"ph1_{it}", tag="ph1", bufs=1)
                ph3 = psum.tile([P, ncap], F32, name=f"ph3_{it}", tag="ph3", bufs=1)
                for ph, woff in ((ph1, 0), (ph3, NKH * P)):
                    for kh in range(NKH):
                        lhsT = wq[:, woff + kh * P : woff + (kh + 1) * P]
                        for ts in blks:
                            nc.tensor.matmul(
                                ph[:, ts], lhsT, xg_sb[kh][:, ts],
                                start=(kh == 0), stop=(kh == NKH - 1),
                            )

                silu1 = stream.tile([P, ncap], F32, name=f"silu_{it}", tag="silu", bufs=2)
                nc.scalar.activation(silu1[:], ph1[:], mybir.ActivationFunctionType.Silu)
                act = persist.tile([P, ncap], F32R, name=f"act_{it}", tag=f"act_{it}")
                nc.vector.tensor_mul(act[:], silu1[:], ph3[:])
                act_sb.append(act)

                if it >= NI - NH:
                    ht = it - (NI - NH)
                    poa = psum.tile([P, ncap], F32, name=f"poa_{ht}", tag="poa", bufs=2)
                    down_mms(poa, ht, range(HALF), 0, HALF - 1)
                    pa = persist.tile([P, ncap], F32, name=f"poa_sb_{ht}", tag=f"poa_sb_{ht}")
                    nc.vector.tensor_copy(pa[:], poa[:])
                    poa_sb[ht] = pa

            # ---- stage 2: second ii-half per h-tile, add+scale, DMA out ----
            for ht in range(NH):
                po = psum.tile([P, ncap], F32, name=f"po_{ht}", tag="po", bufs=2)
                down_mms(po, ht, range(HALF, NI), HALF, NI - 1)
                outsb = stream.tile([P, ncap], F32, name=f"outsb_{ht}", tag="outsb", bufs=2)
                nc.vector.tensor_add(outsb[:], po[:], poa_sb[ht][:])
                nc.vector.tensor_mul(outsb[:], outsb[:], comb_sb[:])
                nc.sync.dma_start(out=outp[ht * P : (ht + 1) * P, :], in_=outsb[:])

    nc.compile()
    _NC_CACHE[ncap] = nc
    return nc


def _route(x: np.ndarray, gw: np.ndarray) -> np.ndarray:
    """Host router: softmax over expert logits, top-2, renormalize.

    Returns combine [T, E] f32 with zeros for unselected experts.
    """
    logits = x @ gw.T                                   # [T, E]
    logits = logits - logits.max(axis=1, keepdims=True)
    ex = np.exp(logits)
    rw = ex / ex.sum(axis=1, keepdims=True)
    idx = np.argsort(-rw, axis=1, kind="stable")[:, :TOPK]
    v = np.take_along_axis(rw, idx, axis=1)
    v = v / v.sum(axis=1, keepdims=True)
    combine = np.zeros((T, E), np.float32)
    np.put_along_axis(combine, idx, v.astype(np.float32), axis=1)
    return combine


def _pack_weights(wsl: np.ndarray) -> list:
    """wsl: [E, 3*I*H] -> per-expert packed wq [NI, P, WCOLS]."""
    packs = []
    for c in range(N_CORES):
        w1 = wsl[c, : I * H].reshape(I, H)
        w3 = wsl[c, I * H : 2 * I * H].reshape(I, H)
        w2 = wsl[c, 2 * I * H :].reshape(H, I)
        wq = np.empty((NI, P, WCOLS), np.float32)
        # lhsT blocks: wq[it, p, kh*P+m] = w[it*P+m, kh*P+p]
        wq[:, :, : NKH * P] = (
            w1.reshape(NI, P, NKH, P).transpose(0, 3, 2, 1).reshape(NI, P, NKH * P)
        )
        wq[:, :, NKH * P : 2 * NKH * P] = (
            w3.reshape(NI, P, NKH, P).transpose(0, 3, 2, 1).reshape(NI, P, NKH * P)
        )
        # w2 rows: wq[it, p, h] = w2[h, it*P+p]
        wq[:, :, 2 * NKH * P :] = np.ascontiguousarray(w2.T).reshape(NI, P, H)
        packs.append(wq)
    return packs


def prepare_in_maps(index, hidden_states, gate_w, ws):
    x = np.ascontiguousarray(np.asarray(hidden_states, dtype=np.float32))
    li = int(index)
    gw = np.asarray(gate_w, dtype=np.float32)[li]       # [E, H]
    wsl = np.asarray(ws, dtype=np.float32)[li]          # [E, 3*I*H]

    combine = _route(x, gw)
    counts = (combine > 0).sum(axis=0)
    ncap = NCAP if counts.max() <= NCAP else T

    xt = np.ascontiguousarray(x.T)                      # [H, T]
    packs = _pack_weights(wsl)

    in_maps = []
    toks_list = []
    for c in range(N_CORES):
        if ncap == T:
            toks = np.arange(T)
        else:
            toks = np.nonzero(combine[:, c] > 0)[0]
        n_c = len(toks)
        xg = np.zeros((H, ncap), np.float32)
        xg[:, :n_c] = xt[:, toks]
        comb_c = np.zeros((P, ncap), np.float32)
        comb_c[:, :n_c] = combine[toks, c][None, :]
        in_maps.append({"xg": xg, "comb": comb_c, "wq": packs[c]})
        toks_list.append(toks)
    return in_maps, toks_list, ncap


def run_device(in_maps, ncap, **spmd_kwargs):
    nc = build_nc(ncap)
    return run_bass_kernel_spmd(nc, in_maps, list(range(N_CORES)), **spmd_kwargs)


def assemble_output(results, toks_list) -> np.ndarray:
    outT = np.zeros((T, H), np.float32)
    for c in range(N_CORES):
        toks = toks_list[c]
        outT[toks, :] += results[c]["outp"][:, : len(toks)].T
    return outT


def kernel(index, hidden_states, experts_cache, gate_w, ws) -> np.ndarray:
    in_maps, toks_list, ncap = prepare_in_maps(index, hidden_states, gate_w, ws)
    res = run_device(in_maps, ncap)
    return assemble_output(res.results, toks_list)


# revision 9
# speedup vs baseline: 1.0477x; 1.0477x over previous
"""Mixtral MoE layer (T=1024, H=1024, I=2048, E=8, top-2) on 8 Trainium2 cores.

Strategy: token-sparse expert-parallel. The router (softmax + top-2 +
renormalize -> combine[T, E]) runs on host. Core c owns expert c's FFN and
processes only the tokens routed to expert c (on average T*K/E = 256,
padded to a fixed bucket NCAP=384; zero-padded columns contribute nothing).
Host gathers each expert's token columns of x^T (the "token all-to-all"
shard step), the device computes

    outT_c = (w2_c @ (silu(w1_c @ xg) * (w3_c @ xg))) * combine[toks_c, c]

and host scatter-adds the per-expert [H, n_c] panels back into the full
[T, H] output (the unshard step). If any expert overflows the bucket
(never for 8 experts at these sizes unless routing is degenerate), we fall
back to a dense variant: every core processes all T tokens with its
combine column, same scatter-add (toks = arange(T)).

Matmuls run as float32r (TF32-like precision, ~2.5e-4 rel err end to end,
full PE rate for moving dims >= 256). Weights are repacked on host so each
i-tile's w1/w3 lhsT blocks and w2 rows form one contiguous [128, 12KB]
DMA (~1.5 MiB per dma_start, descriptor-efficient).
"""

import os
import sys

sys.path.insert(0, "/opt/trn_rl_repo")

import numpy as np

import concourse.bacc as bacc
import concourse.tile as tile
from concourse import mybir
from concourse.bass_utils import run_bass_kernel_spmd

F32 = mybir.dt.float32
F32R = mybir.dt.float32r

T = 1024   # tokens
H = 1024   # hidden
I = 2048   # intermediate
E = 8      # experts
TOPK = 2
P = 128
NKH = H // P     # 8  h-tiles (up-proj contraction)
NI = I // P      # 16 i-tiles
NH = H // P      # 8  h-tiles (down-proj output)
N_CORES = 8
NCAP = 384       # token bucket per expert (seed-robust: mean 256, std ~14)
WCOLS = 3 * NKH * P  # packed weight row: w1 blocks | w3 blocks | w2 rows

_NC_CACHE = {}


def build_nc(ncap: int):
    if ncap in _NC_CACHE:
        return _NC_CACHE[ncap]

    # moving-operand blocks of <=512 (>=256 keeps float32r at full rate)
    nblk = (ncap + 511) // 512
    blks = []
    for b in range(nblk):
        lo = b * 512
        blks.append(slice(lo, min(lo + 512, ncap)))

    nc = bacc.Bacc(None, target_bir_lowering=False, num_devices=N_CORES)

    xg_in = nc.declare_dram_parameter("xg", [H, ncap], F32, isOutput=False)
    comb_in = nc.declare_dram_parameter("comb", [P, ncap], F32, isOutput=False)
    # per i-tile packed weights: [w1 lhsT (NKH*P) | w3 lhsT (NKH*P) | w2 rows (H)]
    wq_in = nc.declare_dram_parameter("wq", [NI, P, WCOLS], F32, isOutput=False)
    outp = nc.declare_dram_parameter("outp", [H, ncap], F32, isOutput=True)

    with tile.TileContext(nc) as tc:
        with (
            tc.tile_pool(name="persist", bufs=1) as persist,
            tc.tile_pool(name="stream", bufs=2) as stream,
            tc.tile_pool(name="psum", bufs=1, space="PSUM") as psum,
        ):
            # first i-tile's weights ahead of the xg block so PE starts early;
            # split so the w1 blocks (first consumers) land first
            WB = NKH * P
            wq0 = stream.tile([P, WCOLS], F32R, name="wq_0", tag="wq", bufs=4)
            nc.sync.dma_start(out=wq0[:, 0:WB], in_=wq_in[0][:, 0:WB].bitcast(F32R))

            xg_sb = []

            def load_xg(kh):
                t_ = persist.tile([P, ncap], F32R, name=f"xg_{kh}", tag=f"xg_{kh}")
                nc.sync.dma_start(
                    out=t_[:], in_=xg_in[kh * P : (kh + 1) * P, :].bitcast(F32R)
                )
                xg_sb.append(t_)

            for kh in range(3):
                load_xg(kh)
            nc.sync.dma_start(
                out=wq0[:, WB : 2 * WB], in_=wq_in[0][:, WB : 2 * WB].bitcast(F32R)
            )
            for kh in range(3, NKH):
                load_xg(kh)
            nc.sync.dma_start(
                out=wq0[:, 2 * WB :], in_=wq_in[0][:, 2 * WB :].bitcast(F32R)
            )
            comb_sb = persist.tile([P, ncap], F32, name="comb_sb", tag="comb_sb")
            nc.sync.dma_start(out=comb_sb[:], in_=comb_in[:, :])

            w2_sb = []
            act_sb = []

            # ---- stage 1: up-projections + SwiGLU, per i-tile ----
            for it in range(NI):
                if it == 0:
                    wq = wq0
                else:
                    wq = stream.tile([P, WCOLS], F32R, name=f"wq_{it}", tag="wq", bufs=4)
                    nc.sync.dma_start(out=wq[:], in_=wq_in[it].bitcast(F32R))

                # stash w2 rows for stage 2 (the streaming tile gets recycled)
                w2sb = persist.tile([P, H], F32R, name=f"w2sb_{it}", tag=f"w2sb_{it}")
                nc.vector.tensor_copy(w2sb[:], wq[:, 2 * NKH * P :].bitcast(F32))
                w2_sb.append(w2sb)

                ph1 = psum.tile([P, ncap], F32, name=f"ph1_{it}", tag="ph1", bufs=1)
                ph3 = psum.tile([P, ncap], F32, name=f"ph3_{it}", tag="ph3", bufs=1)
                for ph, woff in ((ph1, 0), (ph3, NKH * P)):
                    for kh in range(NKH):
                        lhsT = wq[:, woff + kh * P : woff + (kh + 1) * P]
                        for ts in blks:
                            nc.tensor.matmul(
                                ph[:, ts], lhsT, xg_sb[kh][:, ts],
                                start=(kh == 0), stop=(kh == NKH - 1),
                            )

                silu1 = stream.tile([P, ncap], F32, name=f"silu_{it}", tag="silu", bufs=2)
                nc.scalar.activation(silu1[:], ph1[:], mybir.ActivationFunctionType.Silu)
                act = persist.tile([P, ncap], F32R, name=f"act_{it}", tag=f"act_{it}")
                nc.vector.tensor_mul(act[:], silu1[:], ph3[:])
                act_sb.append(act)

            # ---- stage 2: down-projection per h-tile, combine scale, DMA out ----
            for ht in range(NH):
                po = psum.tile([P, ncap], F32, name=f"po_{ht}", tag="po", bufs=2)
                for ii in range(NI):
                    lhsT = w2_sb[ii][:, ht * P : (ht + 1) * P]
                    for ts in blks:
                        nc.tensor.matmul(
                            po[:, ts], lhsT, act_sb[ii][:, ts],
                            start=(ii == 0), stop=(ii == NI - 1),
                        )
                outsb = stream.tile([P, ncap], F32, name=f"outsb_{ht}", tag="outsb", bufs=2)
                nc.vector.tensor_mul(outsb[:], po[:], comb_sb[:])
                nc.sync.dma_start(out=outp[ht * P : (ht + 1) * P, :], in_=outsb[:])

    nc.compile()
    _NC_CACHE[ncap] = nc
    return nc


def _route(x: np.ndarray, gw: np.ndarray) -> np.ndarray:
    """Host router: softmax over expert logits, top-2, renormalize.

    Returns combine [T, E] f32 with zeros for unselected experts.
    """
    logits = x @ gw.T                                   # [T, E]
    logits = logits - logits.max(axis=1, keepdims=True)
    ex = np.exp(logits)
    rw = ex / ex.sum(axis=1, keepdims=True)
    idx = np.argsort(-rw, axis=1, kind="stable")[:, :TOPK]
    v = np.take_along_axis(rw, idx, axis=1)
    v = v / v.sum(axis=1, keepdims=True)
    combine = np.zeros((T, E), np.float32)
    np.put_along_axis(combine, idx, v.astype(np.float32), axis=1)
    return combine


def _pack_weights(wsl: np.ndarray) -> list:
    """wsl: [E, 3*I*H] -> per-expert packed wq [NI, P, WCOLS]."""
    packs = []
    for c in range(N_CORES):
        w1 = wsl[c, : I * H].reshape(I, H)
        w3 = wsl[c, I * H : 2 * I * H].reshape(I, H)
        w2 = wsl[c, 2 * I * H :].reshape(H, I)
        wq = np.empty((NI, P, WCOLS), np.float32)
        # lhsT blocks: wq[it, p, kh*P+m] = w[it*P+m, kh*P+p]
        wq[:, :, : NKH * P] = (
            w1.reshape(NI, P, NKH, P).transpose(0, 3, 2, 1).reshape(NI, P, NKH * P)
        )
        wq[:, :, NKH * P : 2 * NKH * P] = (
            w3.reshape(NI, P, NKH, P).transpose(0, 3, 2, 1).reshape(NI, P, NKH * P)
        )
        # w2 rows: wq[it, p, h] = w2[h, it*P+p]
        wq[:, :, 2 * NKH * P :] = np.ascontiguousarray(w2.T).reshape(NI, P, H)
        packs.append(wq)
    return packs


def prepare_in_maps(index, hidden_states, gate_w, ws):
    x = np.ascontiguousarray(np.asarray(hidden_states, dtype=np.float32))
    li = int(index)
    gw = np.asarray(gate_w, dtype=np.float32)[li]       # [E, H]
    wsl = np.asarray(ws, dtype=np.float32)[li]          # [E, 3*I*H]

    combine = _route(x, gw)
    counts = (combine > 0).sum(axis=0)
    ncap = NCAP if counts.max() <= NCAP else T

    xt = np.ascontiguousarray(x.T)                      # [H, T]
    packs = _pack_weights(wsl)

    in_maps = []
    toks_list = []
    for c in range(N_CORES):
        if ncap == T:
            toks = np.arange(T)
        else:
            toks = np.nonzero(combine[:, c] > 0)[0]
        n_c = len(toks)
        xg = np.zeros((H, ncap), np.float32)
        xg[:, :n_c] = xt[:, toks]
        comb_c = np.zeros((P, ncap), np.float32)
        comb_c[:, :n_c] = combine[toks, c][None, :]
        in_maps.append({"xg": xg, "comb": comb_c, "wq": packs[c]})
        toks_list.append(toks)
    return in_maps, toks_list, ncap


def run_device(in_maps, ncap, **spmd_kwargs):
    nc = build_nc(ncap)
    return run_bass_kernel_spmd(nc, in_maps, list(range(N_CORES)), **spmd_kwargs)


def assemble_output(results, toks_list) -> np.ndarray:
    outT = np.zeros((T, H), np.float32)
    for c in range(N_CORES):
        toks = toks_list[c]
        outT[toks, :] += results[c]["outp"][:, : len(toks)].T
    return outT


def kernel(index, hidden_states, experts_cache, gate_w, ws) -> np.ndarray:
    in_maps, toks_list, ncap = prepare_in_maps(index, hidden_states, gate_w, ws)
    res = run_device(in_maps, ncap)
    return assemble_output(res.results, toks_list)
